# revision 16
# baseline (speedup 1.0000x reference)
"""DeepWuKong GCN (3-layer GCNConv + max/mean pool + FFN) on 8 TRN2 NeuronCores.

Strategy (graph-level data parallelism, per sharding hint):
  - 128 graphs -> 16 graphs/core; each graph padded to 512 node slots
    (4 aligned 128-slot blocks), 8192 node slots/core, 65536 global table
    rows.  The z table (bf16) is split into A/B halves by local row
    (rows 0:4096 / 4096:8192 of each core = graphs 0-7 / 8-15), so each
    half-table is 32768 rows (int16-gatherable) and is AllGathered
    separately -- the B AllGather overlaps pass-A gather/compute.
  - Per layer, two passes over the 64 dst blocks: pass A processes edges
    whose src is in the A half (gather rows from shared z_full_A via
    SWDGE, scaled-one-hot PE matmuls accumulate in PSUM, partial saved
    to SBUF bf16); pass B re-injects the partial via an identity matmul,
    adds the B-half edges, then ScalarE applies bias+ReLU producing the
    next h block (bf16, feature-major).  The NEXT layer's linear
    transform (z = h @ Wc, node-major via swapped matmul operands -- no
    transposes) is fused into pass B per block, so the next AllGather
    pair fires mid-pass.
  - Gather calls pack 8 chunks (1024 indices, the SWDGE per-call ucode
    cap) regardless of block boundaries; a block's PSUM accumulation
    group spans calls when its chunk run straddles one.  Per-block
    shortfall vs the cross-core max is padded with row-0 indices whose
    one-hot columns are zero (dstmod -1 / norm 0), keeping the SPMD
    program identical on every core.
  - Segment pooling is per-core local (graph slots contiguous); FFN is
    two tiny matmuls; host stitches the 8 [16,2] outputs.
"""
import sys

sys.path.insert(0, "/opt/trn_rl_repo")

import numpy as np

import concourse.bacc as bacc
import concourse.bass as bass
import concourse.mybir as mybir
import concourse.tile as tile
from concourse.bass_utils import run_bass_kernel_spmd

# ---- problem constants (hardcoded per spec) --------------------------------
N_NODES = 50000
N_EDGES = 600000
N_GRAPHS = 128
D = 128
N_LAYERS = 3
N_CORES = 8
GPC = N_GRAPHS // N_CORES      # 16 graphs per core
GSLOT = 512                    # node slots per graph (4 blocks of 128)
NLOC = GPC * GSLOT             # 8192 node slots per core
NBLK = NLOC // 128             # 64 blocks per core
HBLK = NBLK // 2               # 32 blocks per half
HLOC = NLOC // 2               # 4096 rows per half
TOT_H = N_CORES * HLOC         # 32768 rows per half-table
BPG = GSLOT // 128             # blocks per graph

F32 = mybir.dt.float32
BF16 = mybir.dt.bfloat16
I16 = mybir.dt.int16

# SWDGE: scratch ring is dynamic_dma_scratch_size//16 descriptors; one call
# must stay under it.
DMA_SCRATCH = 16384
N_QUEUES = 4
MAX_IDX_PER_CALL = 1024
CPC = MAX_IDX_PER_CALL // 128    # chunks per gather call
AG_EMIT_DELAY = 4              # blocks past the half boundary before AG emit


# ===========================================================================
# host-side schedule construction
# ===========================================================================
def _build_schedule(x, edge_index, batch):
    x = np.asarray(x, np.float32)
    ei = np.asarray(edge_index).astype(np.int64)
    batch = np.asarray(batch).astype(np.int64)

    counts = np.bincount(batch, minlength=N_GRAPHS)
    assert counts.max() <= GSLOT, f"graph too big: {counts.max()}"

    deg = np.bincount(ei[1], minlength=N_NODES).astype(np.float64) + 1.0
    dis = 1.0 / np.sqrt(deg)

    graph_start = np.zeros(N_GRAPHS + 1, np.int64)
    np.cumsum(counts, out=graph_start[1:])

    # degree-balanced placement of each graph's nodes into its BPG blocks
    newidx = np.full(N_NODES, -1, np.int64)
    for g in range(N_GRAPHS):
        nodes = np.arange(graph_start[g], graph_start[g + 1])
        if len(nodes) == 0:
            continue
        order = np.argsort(-deg[nodes], kind="stable")
        base = (g // GPC) * NLOC + (g % GPC) * GSLOT
        bin_load = np.zeros(BPG)
        bin_fill = np.zeros(BPG, np.int64)
        for n in nodes[order]:
            cand = np.argsort(bin_load, kind="stable")
            for b in cand:
                if bin_fill[b] < 128:
                    break
            newidx[n] = base + b * 128 + bin_fill[b]
            bin_fill[b] += 1
            bin_load[b] += deg[n]
    assert (newidx >= 0).all()

    # real edges only; self-loops are applied on-chip from the resident z
    # tiles (one diagonal one-hot matmul per block), never gathered
    src, dst = ei[0], ei[1]
    w = (dis[src] * dis[dst]).astype(np.float32)
    psrc = newidx[src]
    pdst = newidx[dst]
    score = psrc // NLOC                 # owner core of src
    slrow = psrc % NLOC                  # src local row
    half = (slrow >= HLOC).astype(np.int64)           # A=0 / B=1 by src half
    srow = score * HLOC + (slrow % HLOC)              # row in half-table
    core = pdst // NLOC                  # owner core of dst (edge owner)
    ldst = pdst % NLOC
    blk = ldst // 128

    # per (core, block, half) counts -> common chunk count K and real count
    cnt = np.zeros((N_CORES, NBLK, 2), np.int64)
    np.add.at(cnt, (core, blk, half), 1)
    cntmax = cnt.max(axis=0)                       # [NBLK, 2]
    K = -(-cntmax // 128)                          # [NBLK, 2]
    assert (cnt > 0).all(), "empty (block,half) bucket"
    assert K.max() * 128 <= MAX_IDX_PER_CALL, f"call too big: K={K.max()}"
    NCH = int(K.sum())

    # slot offsets per (block, half); chunk offsets global over both halves
    slot_off = np.zeros((NBLK, 2), np.int64)
    ch_off = np.zeros((NBLK, 2), np.int64)
    acc_a = acc_b = 0
    acc_ch = 0
    for b in range(NBLK):
        slot_off[b, 0] = acc_a
        acc_a += K[b, 0] * 128
        slot_off[b, 1] = acc_b
        acc_b += K[b, 1] * 128
        ch_off[b, 0] = acc_ch
        acc_ch += K[b, 0]
        ch_off[b, 1] = acc_ch
        acc_ch += K[b, 1]
    slots_a, slots_b = int(acc_a), int(acc_b)

    idx_a = np.zeros((N_CORES, slots_a), np.int16)
    idx_b = np.zeros((N_CORES, slots_b), np.int16)
    # cols 0:NCH = gathered chunks; cols NCH:NCH+NBLK = per-block self-loop
    # diagonals ((iota==p) * dis[node]^2)
    dstmod = np.full((N_CORES, 128, NCH + NBLK), -1.0, np.float32)
    normv = np.zeros((N_CORES, 128, NCH + NBLK), np.float32)
    selfn = (dis * dis).astype(np.float32)
    ar = np.arange(128, dtype=np.float32)
    for c in range(N_CORES):
        dstmod[c, :, NCH:] = ar[:, None]
    np.add.at(normv, (newidx // NLOC, newidx % 128, NCH + (newidx % NLOC) // 128),
              selfn)

    # vectorized per-(core,blk,half) slot assignment
    sort = np.lexsort((half, blk, core))
    s_core, s_blk, s_half = core[sort], blk[sort], half[sort]
    s_sr, s_ld, s_w = srow[sort], ldst[sort], w[sort]
    gid = (s_core * NBLK + s_blk) * 2 + s_half
    first = np.ones(len(gid), bool)
    first[1:] = gid[1:] != gid[:-1]
    gstart = np.zeros(len(gid), np.int64)
    idxs_first = np.flatnonzero(first)
    gstart[idxs_first] = idxs_first
    gstart = np.maximum.accumulate(gstart)
    pos = np.arange(len(gid)) - gstart                  # within-group position

    slot = slot_off[s_blk, s_half] + pos
    chcol = ch_off[s_blk, s_half] + pos // 128
    part = pos % 128
    am = s_half == 0
    idx_a[s_core[am], slot[am]] = s_sr[am].astype(np.int16)
    idx_b[s_core[~am], slot[~am]] = s_sr[~am].astype(np.int16)
    dstmod[s_core, part, chcol] = (s_ld % 128).astype(np.float32)
    normv[s_core, part, chcol] = s_w

    # flat per-half chunk list (block-major == slot order); calls pack up to
    # CPC chunks (1024 idx ucode cap), block boundaries fall anywhere inside
    calls = ([], [])
    for hf in range(2):
        flat = []
        for b in range(NBLK):
            for j in range(int(K[b, hf])):
                flat.append((int(ch_off[b, hf]) + j, b, j == 0,
                             j == int(K[b, hf]) - 1))
        for i in range(0, len(flat), CPC):
            calls[hf].append((i * 128, flat[i:i + CPC]))

    def wrap_idx(a):                 # [slots] -> [128, slots/16], 8x replicated
        w16 = a.reshape(-1, 16).T
        return np.tile(w16, (8, 1)).copy()

    idx_a_w = np.stack([wrap_idx(idx_a[c]) for c in range(N_CORES)])
    idx_b_w = np.stack([wrap_idx(idx_b[c]) for c in range(N_CORES)])

    xpad = np.zeros((N_CORES * NLOC, D), np.float32)
    xpad[newidx] = x
    x_fm = np.stack([xpad[c * NLOC:(c + 1) * NLOC].T.copy()
                     for c in range(N_CORES)])

    invcnt = (1.0 / np.maximum(counts, 1)).astype(np.float32)
    invcnt_rep = np.stack([
        np.tile(invcnt[c * GPC:(c + 1) * GPC], (128, 1)) for c in range(N_CORES)
    ]).astype(np.float32)

    def to_bf16(a):
        import ml_dtypes
        return a.astype(ml_dtypes.bfloat16)

    return dict(
        K=K, NCH=NCH, cntmax=cntmax, calls=calls,
        slots_a=slots_a, slots_b=slots_b,
        slot_off=slot_off, ch_off=ch_off,
        idx_a=idx_a_w, idx_b=idx_b_w,
        dstmod=dstmod, normv=normv,
        x_fm=to_bf16(x_fm), invcnt_rep=invcnt_rep,
    )


# ===========================================================================
# device kernel
# ===========================================================================
def _build_kernel(sch):
    K, cntmax = sch["K"], sch["cntmax"]
    slot_off, ch_off = sch["slot_off"], sch["ch_off"]
    calls = sch["calls"]
    NCH = sch["NCH"]
    NA16 = sch["slots_a"] // 16
    NB16 = sch["slots_b"] // 16

    nc = bacc.Bacc(
        "TRN2",
        target_bir_lowering=False,
        debug=False,
        num_devices=N_CORES,
        num_swdge_queues=N_QUEUES,
        dynamic_dma_scratch_size=DMA_SCRATCH,
    )

    xfm_d = nc.dram_tensor("xfm", [128, NLOC], BF16, kind="ExternalInput")
    wc_d = nc.dram_tensor("wc", [N_LAYERS, 128, 128], BF16, kind="ExternalInput")
    bct_d = nc.dram_tensor("bct", [128, N_LAYERS], F32, kind="ExternalInput")
    wffn_d = nc.dram_tensor("wffn", [256, 128], F32, kind="ExternalInput")
    bffnt_d = nc.dram_tensor("bffnt", [128, 1], F32, kind="ExternalInput")
    wfin_d = nc.dram_tensor("wfin", [128, 2], F32, kind="ExternalInput")
    bfinr_d = nc.dram_tensor("bfinr", [GPC, 2], F32, kind="ExternalInput")
    idxa_d = nc.dram_tensor("idxa", [128, NA16], I16, kind="ExternalInput")
    idxb_d = nc.dram_tensor("idxb", [128, NB16], I16, kind="ExternalInput")
    dstmod_d = nc.dram_tensor("dstmod", [128, NCH + NBLK], F32, kind="ExternalInput")
    normv_d = nc.dram_tensor("normv", [128, NCH + NBLK], F32, kind="ExternalInput")
    invc_d = nc.dram_tensor("invc", [128, GPC], F32, kind="ExternalInput")
    iota_d = nc.dram_tensor("iota", [128, 128], BF16, kind="ExternalInput")
    ident_d = nc.dram_tensor("ident", [128, 128], BF16, kind="ExternalInput")
    out_d = nc.dram_tensor("out", [GPC, 2], F32, kind="ExternalOutput")

    RG = [list(range(N_CORES))]

    with tile.TileContext(nc) as tc:
        with (
            tc.tile_pool(name="consts", bufs=1) as consts,
            tc.tile_pool(name="hpool", bufs=2) as hpool,
            tc.tile_pool(name="zpool", bufs=NBLK) as zpool,
            tc.tile_pool(name="gapool", bufs=4) as gapool,
            tc.tile_pool(name="gbpool", bufs=4) as gbpool,
            tc.tile_pool(name="ohpool", bufs=8) as ohpool,
            tc.tile_pool(name="partpool", bufs=NBLK) as partpool,
            tc.tile_pool(name="spool", bufs=1) as spool,
            tc.tile_pool(name="psz", bufs=2, space="PSUM") as psz,
            tc.tile_pool(name="psaggA", bufs=2, space="PSUM") as psaggA,
            tc.tile_pool(name="psaggB", bufs=2, space="PSUM") as psaggB,
            tc.tile_pool(name="psfin", bufs=1, space="PSUM") as psfin,
            tc.tile_pool(name="dram", bufs=1, space="DRAM") as dram,
        ):
            # ---- load constants -------------------------------------------
            wc_sb = consts.tile([128, N_LAYERS, 128], BF16)
            nc.sync.dma_start(wc_sb[:], wc_d[:].rearrange("l p f -> p l f"))
            bct_sb = consts.tile([128, N_LAYERS], F32)
            nc.sync.dma_start(bct_sb[:], bct_d[:])
            wffn_sb = consts.tile([128, 2, 128], F32)
            nc.sync.dma_start(
                wffn_sb[:], wffn_d[:].rearrange("(h p) f -> p h f", p=128))
            bffnt_sb = consts.tile([128, 1], F32)
            nc.sync.dma_start(bffnt_sb[:], bffnt_d[:])
            wfin_sb = consts.tile([128, 2], F32)
            nc.sync.dma_start(wfin_sb[:], wfin_d[:])
            bfinr_sb = consts.tile([GPC, 2], F32)
            nc.sync.dma_start(bfinr_sb[:], bfinr_d[:])
            idxa_sb = consts.tile([128, NA16], I16)
            nc.sync.dma_start(idxa_sb[:], idxa_d[:])
            idxb_sb = consts.tile([128, NB16], I16)
            nc.sync.dma_start(idxb_sb[:], idxb_d[:])
            dstmod_sb = consts.tile([128, NCH + NBLK], F32)
            nc.sync.dma_start(dstmod_sb[:], dstmod_d[:])
            normv_sb = consts.tile([128, NCH + NBLK], F32)
            nc.sync.dma_start(normv_sb[:], normv_d[:])
            invc_sb = consts.tile([128, GPC], F32)
            nc.sync.dma_start(invc_sb[:], invc_d[:])
            iota_sb = consts.tile([128, 128], BF16)
            nc.sync.dma_start(iota_sb[:], iota_d[:])
            ident_sb = consts.tile([128, 128], BF16)
            nc.sync.dma_start(ident_sb[:], ident_d[:])

            x_sb = consts.tile([128, NLOC], BF16)
            nc.sync.dma_start(x_sb[:], xfm_d[:])

            # ---- helpers ---------------------------------------------------
            ztiles = [None] * NBLK

            def transform_block(h_fm_slice, l, b, zown_a, zown_b, uid):
                """z[b*128:(b+1)*128, :] = h_blockT @ Wc[l]; node-major via
                swapped operands, no transpose needed.  The z tile stays
                resident in SBUF for the next layer's self-loop matmul."""
                zp = psz.tile([128, 128], F32, tag="zps", name=f"zps{uid}")
                nc.tensor.matmul(zp[:], h_fm_slice, wc_sb[:, l, :],
                                 start=True, stop=True)
                zs = zpool.tile([128, 128], BF16, tag="zsb", name=f"zsb{uid}")
                nc.vector.tensor_copy(zs[:], zp[:])
                ztiles[b] = zs
                if b < HBLK:
                    nc.sync.dma_start(zown_a[b * 128:(b + 1) * 128, :], zs[:])
                else:
                    bb = b - HBLK
                    nc.sync.dma_start(zown_b[bb * 128:(bb + 1) * 128, :], zs[:])

            def gather_call(zfull, idx_sb, hf, l, ci, q):
                slot0, grp = calls[hf][ci]
                nch = len(grp)
                pool = gapool if hf == 0 else gbpool
                g = pool.tile([128, CPC, 128], BF16,
                              tag=("ga" if hf == 0 else "gb"),
                              name=f"g{l}_{hf}_{ci}")
                c0 = slot0 // 16
                nc.gpsimd.dma_gather(
                    g[:, :nch, :], zfull[:],
                    idx_sb[:, c0:c0 + nch * 8],
                    num_idxs=nch * 128, num_idxs_reg=nch * 128,
                    elem_size=128, queue_num=q % N_QUEUES,
                )
                return g

            def one_hot(ch, l):
                oh = ohpool.tile([128, 128], BF16, tag="oh",
                                 name=f"oh{l}_{ch}")
                nc.vector.tensor_scalar(
                    oh[:], iota_sb[:],
                    dstmod_sb[:, ch:ch + 1],
                    normv_sb[:, ch:ch + 1],
                    mybir.AluOpType.is_equal,
                    mybir.AluOpType.mult,
                )
                return oh

            # ---- layer 0 z from x -----------------------------------------
            zown_a = [None] * (N_LAYERS + 1)
            zown_b = [None] * (N_LAYERS + 1)
            zfull_a = [None] * (N_LAYERS + 1)
            zfull_b = [None] * (N_LAYERS + 1)

            def new_ztiles(l):
                zown_a[l] = dram.tile([HLOC, 128], BF16, tag="zowna", bufs=2,
                                      name=f"zowna{l}")
                zown_b[l] = dram.tile([HLOC, 128], BF16, tag="zownb", bufs=2,
                                      name=f"zownb{l}")
                zfull_a[l] = dram.tile([TOT_H, 128], BF16, tag="zfulla", bufs=2,
                                       addr_space="Shared", name=f"zfulla{l}")
                zfull_b[l] = dram.tile([TOT_H, 128], BF16, tag="zfullb", bufs=2,
                                       addr_space="Shared", name=f"zfullb{l}")

            def emit_ag(l, hf):
                zo = zown_a[l] if hf == 0 else zown_b[l]
                zf = zfull_a[l] if hf == 0 else zfull_b[l]
                nc.gpsimd.collective_compute(
                    "AllGather", mybir.AluOpType.bypass,
                    replica_groups=RG,
                    ins=[zo[:].opt()],
                    outs=[zf[:].opt()],
                )

            new_ztiles(0)
            for b in range(NBLK):
                transform_block(x_sb[:, b * 128:(b + 1) * 128], 0, b,
                                zown_a[0], zown_b[0], f"x{b}")
                if b == HBLK - 1:
                    emit_ag(0, 0)
            emit_ag(0, 1)
            tc.no_sync_barrier()

            # ---- layers ----------------------------------------------------
            mx = spool.tile([128, GPC], F32)
            sm = spool.tile([128, GPC], F32)
            h_cur = None
            for l in range(N_LAYERS):
                # pass A: accumulate A-half messages, stash partials (bf16)
                parts = [None] * NBLK
                ps = None
                for ci in range(len(calls[0])):
                    g = gather_call(zfull_a[l], idxa_sb, 0, l, ci, ci)
                    for jj, (ch, b, first, last) in enumerate(calls[0][ci][1]):
                        if first:
                            ps = psaggA.tile([128, 128], F32, tag="aggpsA",
                                             name=f"aggA{l}_{b}")
                            ohs = one_hot(NCH + b, l)
                            nc.tensor.matmul(ps[:], ztiles[b][:], ohs[:],
                                             start=True, stop=False)
                        oh = one_hot(ch, l)
                        nc.tensor.matmul(ps[:], g[:, jj, :], oh[:],
                                         start=False, stop=last)
                        if last:
                            pt = partpool.tile([128, 128], BF16, tag="part",
                                               name=f"part{l}_{b}")
                            nc.scalar.activation(
                                pt[:], ps[:],
                                mybir.ActivationFunctionType.Copy)
                            parts[b] = pt

                tc.no_sync_barrier()
                h_nxt = hpool.tile([128, NLOC], BF16, tag="h", name=f"h{l + 1}")
                if l < N_LAYERS - 1:
                    new_ztiles(l + 1)

                # pass B: re-inject partial, add B-half, bias+relu, fused
                # next-layer transform
                ps = None
                for ci in range(len(calls[1])):
                    g = gather_call(zfull_b[l], idxb_sb, 1, l, ci, ci)
                    for jj, (ch, b, first, last) in enumerate(calls[1][ci][1]):
                        if first:
                            ps = psaggB.tile([128, 128], F32, tag="aggpsB",
                                             name=f"aggB{l}_{b}")
                            nc.tensor.matmul(ps[:], ident_sb[:], parts[b][:],
                                             start=True, stop=False)
                        oh = one_hot(ch, l)
                        nc.tensor.matmul(ps[:], g[:, jj, :], oh[:],
                                         start=False, stop=last)
                        if not last:
                            continue
                        nc.scalar.activation(
                            h_nxt[:, b * 128:(b + 1) * 128], ps[:],
                            mybir.ActivationFunctionType.Relu,
                            bias=bct_sb[:, l:l + 1])
                        if l < N_LAYERS - 1:
                            transform_block(h_nxt[:, b * 128:(b + 1) * 128],
                                            l + 1, b, zown_a[l + 1],
                                            zown_b[l + 1], f"{l + 1}_{b}")
                            if b == HBLK - 1 + AG_EMIT_DELAY:
                                tc.no_sync_barrier()
                                emit_ag(l + 1, 0)
                        elif b % BPG == BPG - 1:
                            gg = b // BPG
                            nc.vector.tensor_reduce(
                                mx[:, gg:gg + 1],
                                h_nxt[:, gg * GSLOT:(gg + 1) * GSLOT],
                                mybir.AxisListType.X, mybir.AluOpType.max)
                            nc.vector.tensor_reduce(
                                sm[:, gg:gg + 1],
                                h_nxt[:, gg * GSLOT:(gg + 1) * GSLOT],
                                mybir.AxisListType.X, mybir.AluOpType.add)
                tc.no_sync_barrier()
                if l < N_LAYERS - 1:
                    emit_ag(l + 1, 1)
                h_cur = h_nxt

            # ---- pooling tail + FFN (per-graph reduces fused into pass B
            # of the last layer) ---------------------------------------------
            mean = spool.tile([128, GPC], F32)
            nc.vector.tensor_tensor(
                mean[:], sm[:], invc_sb[:], mybir.AluOpType.mult)

            p1 = psfin.tile([128, GPC], F32, tag="p1")
            nc.tensor.matmul(p1[:], wffn_sb[:, 0, :], mx[:],
                             start=True, stop=False)
            nc.tensor.matmul(p1[:], wffn_sb[:, 1, :], mean[:],
                             start=False, stop=True)
            o1 = spool.tile([128, GPC], F32)
            nc.scalar.activation(
                o1[:], p1[:], mybir.ActivationFunctionType.Relu,
                bias=bffnt_sb[:, 0:1])

            p2 = psfin.tile([GPC, 2], F32, tag="p2")
            nc.tensor.matmul(p2[:], o1[:], wfin_sb[:], start=True, stop=True)
            osb = spool.tile([GPC, 2], F32)
            nc.vector.tensor_tensor(
                osb[:], p2[:], bfinr_sb[:], mybir.AluOpType.add)
            nc.sync.dma_start(out_d[:], osb[:])

    nc.compile()
    return nc


# ===========================================================================
# entry point
# ===========================================================================
_CACHE = {}


def kernel(x, Wc, bc, W_ffn, b_ffn, W_fin, b_fin, edge_index, batch):
    import ml_dtypes
    x = np.ascontiguousarray(np.asarray(x, np.float32))
    Wc = np.ascontiguousarray(np.asarray(Wc, np.float32))
    bc = np.ascontiguousarray(np.asarray(bc, np.float32))
    W_ffn = np.ascontiguousarray(np.asarray(W_ffn, np.float32))
    b_ffn = np.ascontiguousarray(np.asarray(b_ffn, np.float32))
    W_fin = np.ascontiguousarray(np.asarray(W_fin, np.float32))
    b_fin = np.ascontiguousarray(np.asarray(b_fin, np.float32))

    sch = _build_schedule(x, edge_index, batch)

    key = (sch["NCH"], sch["slots_a"], sch["slots_b"], tuple(sch["K"].ravel()),
           tuple(sch["cntmax"].ravel()))
    if key not in _CACHE:
        _CACHE.clear()
        _CACHE[key] = _build_kernel(sch)
    nc = _CACHE[key]

    iota = np.tile(np.arange(128, dtype=np.float32)[None, :],
                   (128, 1)).astype(ml_dtypes.bfloat16)
    ident = np.eye(128, dtype=np.float32).astype(ml_dtypes.bfloat16)
    wc_bf = Wc.astype(ml_dtypes.bfloat16)
    bct = bc.T.copy()                       # [128, 3]
    bffnt = b_ffn[:, None].copy()           # [128, 1]
    bfinr = np.tile(b_fin[None, :], (GPC, 1)).astype(np.float32)

    in_maps = []
    for c in range(N_CORES):
        in_maps.append({
            "xfm": sch["x_fm"][c],
            "wc": wc_bf, "bct": bct, "wffn": W_ffn, "bffnt": bffnt,
            "wfin": W_fin, "bfinr": bfinr,
            "idxa": sch["idx_a"][c], "idxb": sch["idx_b"][c],
            "dstmod": sch["dstmod"][c], "normv": sch["normv"][c],
            "invc": sch["invcnt_rep"][c],
            "iota": iota, "ident": ident,
        })

    _CACHE["in_maps"] = in_maps
    res = run_bass_kernel_spmd(nc, in_maps, core_ids=list(range(N_CORES)))
    out = np.concatenate([res.results[c]["out"] for c in range(N_CORES)], 0)
    return out.astype(np.float32)


def timed_run(inputs=None):
    """Re-run the cached compiled kernel with profiling; returns exec ns."""
    import time
    nc = next(v for k, v in _CACHE.items() if k != "in_maps")
    in_maps = _CACHE["in_maps"]
    walls = []
    for _ in range(3):
        t0 = time.time()
        run_bass_kernel_spmd(nc, in_maps, core_ids=list(range(N_CORES)))
        walls.append(time.time() - t0)
    print(f"warm re-run walls: {[f'{w*1e3:.1f}ms' for w in walls]}")
    try:
        res = run_bass_kernel_spmd(
            nc, in_maps, core_ids=list(range(N_CORES)), trace=True)
        if res.exec_time_ns is not None:
            return res.exec_time_ns
    except Exception as e:
        print(f"(ntff profiling unavailable: {type(e).__name__}; "
              f"reporting warm wall-clock upper bound)")
    return int(min(walls) * 1e9)


if __name__ == "__main__":
    rng = np.random.default_rng(0)
    x = rng.standard_normal((N_NODES, D), dtype=np.float32)
    ei = rng.integers(0, N_NODES, (2, N_EDGES)).astype(np.int64)
    batch = np.sort(rng.integers(0, N_GRAPHS, N_NODES)).astype(np.int64)
    Wc = rng.standard_normal((3, D, D), dtype=np.float32) * 0.05
    out = kernel(x, Wc, np.zeros((3, D), np.float32),
                 rng.standard_normal((2 * D, D), dtype=np.float32) * 0.05,
                 np.zeros((D,), np.float32),
                 rng.standard_normal((D, 2), dtype=np.float32) * 0.05,
                 np.zeros((2,), np.float32), ei, batch)
    print(out.shape, out[:4])
